# revision 8
# baseline (speedup 1.0000x reference)
"""Trainium2 Bass kernel for GaussianProcessEmbeddingHead.

The reference computes:
    mu     = x @ W_mu.T + b_mu                      (B,N,E)
    sigma  = exp(0.5*(x @ W_logvar.T + b_logvar))   (B,N,E)
    K      = RBF kernel matrix (B,N,N)  -- only its DIAGONAL is used,
             and dist_ii == 0 exactly, so cov_diag == 1 and the (B,N,N)
             work is mathematically dead. sigma_adjusted == sigma.
    return (mu, sigma_adjusted)

Strategy: data-parallel over batch B=8, one batch element per NeuronCore.
Per core: two linear heads over x_b [2048,1024] in bf16. The PE streams
one output column per cycle, so the floor is
   2 heads * (2048*512 outputs / 128 lanes) * (1024/128 k-tiles)
   = 131072 cycles ~= 54.6 us @ 2.4 GHz.

Schedule (v5) — built around the measured DMA cost model:
   queue_time ~ 22ns x n_descriptors (one descriptor per partition-row)
   + data time, and every DMA's completion semaphore lands ~2us after
   its last byte (write-receipt round trip).  Queues share the SDMA
   fabric; per-partition-run size only matters through descriptor count.
So:
 - Every load moves a whole tensor's [p, full-width] rows (max run
   size), PARTITION-SPLIT: rows 0:64 on the sync queue, 64:128 on the
   scalar queue.  Halves use disjoint SDMA engine sets, so each tensor
   lands in ~1.5us of queue time + 2us completion.  Order: wlv, x_c0,
   x_c1, wmu, x_c2, x_c3 — everything is resident by ~13us with zero
   mid-stream dependence.  gpsimd (SWDGE) carries only the tiny biases.
 - lv head runs chunk-major (kt-outer, 4 PSUM banks in parallel) so
   compute starts as soon as wlv+x_c0 land (~5us); its four full-width
   [128, 2048] eb stores (128 descriptors each) issue at lv end, split
   across both queues.
 - mu head runs EB-MAJOR (kt-inner groups): each eb finishes 6.8us
   apart, and its [128, 2048] store issues immediately, partition-split
   across both queues (~1.4us each half).  Only the final tapered
   (256/128/128) eb3 group's store trails the last matmul: the
   post-compute tail is one parallel half-store pair + completion
   (~3.5us) instead of the old 11us serialized store drain.
 - Warmup matmuls (32) keep the PE busy through the DMA lead-in so the
   HAM clock gate reaches 2.4 GHz with no re-throttle gap.
 - Epilogues: one op per PSUM tile with the bias fed through the
   per-partition port:
     sigma = Exp(PSUM * 0.5 + 0.5*b_lv[e])  on the Scalar engine
     mu    = PSUM + b_mu[e]                 on the Vector engine
   both writing bf16 into [128, 2048] out tiles ([p][eb][c][t] DRAM
   layout); host un-transposes and upcasts.
"""
import os
import sys

import numpy as np

try:
    import concourse.bass as bass  # noqa: F401
except Exception:  # pragma: no cover - path fallback for fresh dirs
    for p in ("/opt/trn_rl_repo", os.path.expanduser("~/.axon_site/_ro/trn_rl_repo")):
        if os.path.isdir(p) and p not in sys.path:
            sys.path.insert(0, p)
    import concourse.bass as bass

import ml_dtypes
import concourse.mybir as mybir
from concourse import bacc
from concourse.bass_utils import run_bass_kernel_spmd
from concourse.tile import TileContext

B, N, D, E = 8, 2048, 1024, 512
P = 128
KT = D // P          # 8 k-tiles
EB = E // P          # 4 embedding blocks
TC = N // 512        # 4 token chunks of 512
F32, BF16 = mybir.dt.float32, mybir.dt.bfloat16

_NC = None


def _build():
    nc = bacc.Bacc()
    # x packed on host as [p][c][kt][t] -> [P, KT*N]
    xP = nc.declare_dram_parameter("xP", [P, KT * N], BF16, isOutput=False)
    # weights packed as [p][kt][e] -> [P, KT*E]
    wlv = nc.declare_dram_parameter("wlv", [P, E * KT], BF16, isOutput=False)
    wmu = nc.declare_dram_parameter("wmu", [P, E * KT], BF16, isOutput=False)
    # biases arranged [P, EB]: element (p, eb) = bias[eb*128 + p]
    bmu = nc.declare_dram_parameter("bmu", [P, EB], F32, isOutput=False)
    blv = nc.declare_dram_parameter("blv", [P, EB], F32, isOutput=False)  # 0.5*b
    # outputs packed [p][eb][c][t]: element (p, eb*N + c*512 + t) =
    # head[c*512 + t, eb*128 + p]
    muT = nc.declare_dram_parameter("muT", [P, EB * N], BF16, isOutput=True)
    sgT = nc.declare_dram_parameter("sgT", [P, EB * N], BF16, isOutput=True)

    with TileContext(nc) as tc:
        with (
            tc.tile_pool(name="const", bufs=1) as cpool,
            tc.tile_pool(name="out", bufs=8) as opool,
            tc.tile_pool(name="psA", bufs=4, space="PSUM") as psA,
            tc.tile_pool(name="psB", bufs=4, space="PSUM") as psB,
        ):
            x_sb = [
                cpool.tile([P, KT, 512], BF16, name=f"x_sb{c}") for c in range(TC)
            ]
            wlv_sb = cpool.tile([P, KT, E], BF16)
            wmu_sb = cpool.tile([P, KT, E], BF16)
            blv_sb = cpool.tile([P, EB], F32)
            bmu_sb = cpool.tile([P, EB], F32)
            warm = cpool.tile([P, P], BF16)

            wlv_r = wlv[:, :].rearrange("p (kt e) -> p kt e", kt=KT)
            wmu_r = wmu[:, :].rearrange("p (kt e) -> p kt e", kt=KT)

            def xslab(c):
                off = c * 512 * KT
                return xP[:, off : off + 512 * KT].rearrange(
                    "p (kt t) -> p kt t", kt=KT
                )

            # Warmup: PE busy continuously from ~0.2us so the HAM clock
            # gate flips to 2.4 GHz with no re-throttle gap before the
            # real stream.
            nc.vector.memset(warm, 0)
            wps = psA.tile([P, P], F32, tag="ps", name="warmps")
            for i in range(32):
                nc.tensor.matmul(
                    wps, warm[:, :], warm[:, :], start=(i == 0), stop=(i == 31)
                )

            # --- loads: partition-split halves, full-width rows -------
            def load_split(dst, src):
                nc.sync.dma_start(out=dst[0:64], in_=src[0:64])
                nc.scalar.dma_start(out=dst[64:P], in_=src[64:P])

            load_split(wlv_sb, wlv_r)
            load_split(x_sb[0], xslab(0))
            load_split(x_sb[1], xslab(1))
            load_split(wmu_sb, wmu_r)
            load_split(x_sb[2], xslab(2))
            load_split(x_sb[3], xslab(3))
            nc.gpsimd.dma_start(out=blv_sb, in_=blv[:, :])
            nc.gpsimd.dma_start(out=bmu_sb, in_=bmu[:, :])

            EXP = mybir.ActivationFunctionType.Exp

            def epilogue(hname, bias_sb, eb, ps, ot, o0, ow):
                """PSUM -> bf16 slice [o0:o0+ow) of the [P, N] out tile."""
                osl = ot[:, o0 : o0 + ow]
                if hname == "lv":
                    nc.scalar.activation(
                        osl, ps, EXP, bias=bias_sb[:, eb : eb + 1], scale=0.5
                    )
                else:
                    nc.vector.tensor_scalar_add(osl, ps, bias_sb[:, eb : eb + 1])

            def store_split(outdram, eb, ot):
                """Store one full-width [P, N] eb tile as two parallel
                partition-halves on the two HWDGE queues."""
                ds = slice(eb * N, (eb + 1) * N)
                nc.sync.dma_start(out=outdram[0:64, ds], in_=ot[0:64, :])
                nc.scalar.dma_start(out=outdram[64:P, ds], in_=ot[64:P, :])

            # --- lv head: chunk-major, kt-outer -----------------------
            ot_lv = [
                opool.tile([P, N], BF16, tag="o", name=f"o_lv{eb}")
                for eb in range(EB)
            ]
            for c in range(TC):
                pool = [psA, psB][c % 2]
                pss = [
                    pool.tile([P, 512], F32, tag="ps", name=f"ps_lv{c}{eb}")
                    for eb in range(EB)
                ]
                for kt in range(KT):
                    for eb in range(EB):
                        nc.tensor.matmul(
                            pss[eb],
                            wlv_sb[:, kt, eb * P : (eb + 1) * P],
                            x_sb[c][:, kt, :],
                            start=(kt == 0),
                            stop=(kt == KT - 1),
                        )
                for eb in range(EB):
                    epilogue("lv", blv_sb, eb, pss[eb], ot_lv[eb], c * 512, 512)
            for eb in range(EB):
                store_split(sgT, eb, ot_lv[eb])

            # --- mu head: EB-MAJOR, kt-inner; store each eb as it
            # finishes so only eb3's store trails the last matmul ------
            for eb in range(EB):
                ot = opool.tile([P, N], BF16, tag="o", name=f"o_mu{eb}")
                for c in range(TC):
                    pieces = (
                        [(0, 512)]
                        if not (eb == EB - 1 and c == TC - 1)
                        else [(0, 256), (256, 128), (384, 128)]
                    )
                    for o0, ow in pieces:
                        ps = psA.tile(
                            [P, ow], F32, tag="ps", name=f"ps_mu{eb}{c}_{o0}"
                        )
                        for kt in range(KT):
                            nc.tensor.matmul(
                                ps,
                                wmu_sb[:, kt, eb * P : (eb + 1) * P],
                                x_sb[c][:, kt, o0 : o0 + ow],
                                start=(kt == 0),
                                stop=(kt == KT - 1),
                            )
                        epilogue("mu", bmu_sb, eb, ps, ot, c * 512 + o0, ow)
                store_split(muT, eb, ot)
    nc.compile()
    return nc


def _pack_x(xb):
    """xb [N, D] f32 -> [P, KT*N] bf16 packed as [p][c][kt][t]."""
    xt = xb.T.astype(ml_dtypes.bfloat16).reshape(KT, P, TC, 512)  # [kt, p, c, t]
    return np.ascontiguousarray(xt.transpose(1, 2, 0, 3).reshape(P, KT * N))


def _pack_w(W):
    """W [E, D] f32 -> [P, KT*E] bf16 packed as [p][kt][e]."""
    wt = W.astype(np.float32).T.astype(ml_dtypes.bfloat16)   # [D, E]
    v = wt.reshape(KT, P, E)
    return np.ascontiguousarray(v.transpose(1, 0, 2).reshape(P, KT * E))


def _unpack_out(a):
    """[P, EB*N] bf16 packed [p][eb][c][t] -> [N, E] f32."""
    v = a.reshape(P, EB, N)                      # [p, eb, n]
    return np.ascontiguousarray(v.transpose(2, 1, 0).reshape(N, E)).astype(np.float32)


def run(x, W_mu, b_mu, W_logvar, b_logvar, trace=False, **trace_kwargs):
    global _NC
    if _NC is None:
        _NC = _build()

    x = np.asarray(x, dtype=np.float32)
    wlv_h = _pack_w(np.asarray(W_logvar))
    wmu_h = _pack_w(np.asarray(W_mu))
    bmu_h = np.ascontiguousarray(np.asarray(b_mu, dtype=np.float32).reshape(EB, P).T)
    blv_h = np.ascontiguousarray(
        (0.5 * np.asarray(b_logvar, dtype=np.float32)).reshape(EB, P).T
    )

    in_maps = [
        {
            "xP": _pack_x(x[b]),
            "wlv": wlv_h,
            "wmu": wmu_h,
            "bmu": bmu_h,
            "blv": blv_h,
        }
        for b in range(B)
    ]
    res = run_bass_kernel_spmd(
        _NC, in_maps, core_ids=list(range(B)), trace=trace, **trace_kwargs
    )
    mu = np.stack([_unpack_out(res.results[b]["muT"]) for b in range(B)])
    sigma = np.stack([_unpack_out(res.results[b]["sgT"]) for b in range(B)])
    return (mu, sigma), res


def kernel(x, W_mu, b_mu, W_logvar, b_logvar):
    (mu, sigma), _ = run(x, W_mu, b_mu, W_logvar, b_logvar, trace=False)
    return mu, sigma


# revision 9
# speedup vs baseline: 1.0190x; 1.0190x over previous
"""Trainium2 Bass kernel for GaussianProcessEmbeddingHead.

The reference computes:
    mu     = x @ W_mu.T + b_mu                      (B,N,E)
    sigma  = exp(0.5*(x @ W_logvar.T + b_logvar))   (B,N,E)
    K      = RBF kernel matrix (B,N,N)  -- only its DIAGONAL is used,
             and dist_ii == 0 exactly, so cov_diag == 1 and the (B,N,N)
             work is mathematically dead. sigma_adjusted == sigma.
    return (mu, sigma_adjusted)

Strategy: data-parallel over batch B=8, one batch element per NeuronCore.
Per core: two linear heads over x_b [2048,1024] in bf16. The PE streams
one output column per cycle, so the floor is
   2 heads * (2048*512 outputs / 128 lanes) * (1024/128 k-tiles)
   = 131072 cycles ~= 54.6 us @ 2.4 GHz.

Schedule (v6) — measured DMA laws this is built around:
  * per-queue throughput ~95-105 B/ns when two queues run, ~200 solo;
    gpsimd (SWDGE) adds ~130 on top but starves the HWDGE queues.
  * every DMA's completion semaphore lands ~2us after its last byte
    for 128-partition transfers — but ~4.5us for partition-SPLIT
    (64-row) transfers, so fine-graining is done by k-tile PAIRS and
    column halves, never by partitions.
So:
 - OPENING (the PE-start critical path): x_c0 and wlv stream as
   interleaved kt-pairs on the two HWDGE queues (sync: w01,x23,w45,x67;
   scalar: x01,w23,x45,w67).  Chunk 0 runs kt-OUTER and consumes each
   pair as it lands (~1 pair / 1.7us of compute), so compute starts at
   ~5.5us and chunk 0 co-completes with its own load.
 - x_c1 rides gpsimd, gated behind the first wlv pair (tiny copy) so
   its SWDGE burst doesn't slow the pairs that gate compute start;
   x_c2/x_c3 are gated behind x_c1, needed only at ~26/33us.  wmu
   follows the opening on the HWDGE queues as two kt-half slabs.
 - lv head: chunk-major (kt-outer, 4 PSUM banks); mu head: EB-MAJOR
   (kt-inner groups) so each eb's output completes 6.8us apart.
 - Stores are [128, 1024] column-halves (2 KB descriptors, full 128
   partitions): each (head, eb) stores chunks 01 as soon as c1's
   epilogue lands and chunks 23 at eb end, alternating queues.  After
   the last (tapered 256/128/128) matmul group only ONE 256 KB store
   remains -> post-compute tail ~5us instead of the 11-13us store
   drains of earlier schedules.
 - Warmup matmuls (32) keep the PE busy through the DMA lead-in so the
   HAM clock gate reaches 2.4 GHz with no re-throttle gap.
 - Epilogues: one op per PSUM tile, bias via the per-partition port:
     sigma = Exp(PSUM * 0.5 + 0.5*b_lv[e])  on the Scalar engine
     mu    = PSUM + b_mu[e]                 on the Vector engine
   both writing bf16 into [128, 2048] out tiles ([p][eb][c][t] DRAM
   layout); host un-transposes and upcasts.
"""
import os
import sys

import numpy as np

try:
    import concourse.bass as bass  # noqa: F401
except Exception:  # pragma: no cover - path fallback for fresh dirs
    for p in ("/opt/trn_rl_repo", os.path.expanduser("~/.axon_site/_ro/trn_rl_repo")):
        if os.path.isdir(p) and p not in sys.path:
            sys.path.insert(0, p)
    import concourse.bass as bass

import ml_dtypes
import concourse.mybir as mybir
from concourse import bacc
from concourse.bass_utils import run_bass_kernel_spmd
from concourse.tile import TileContext

B, N, D, E = 8, 2048, 1024, 512
P = 128
KT = D // P          # 8 k-tiles
EB = E // P          # 4 embedding blocks
TC = N // 512        # 4 token chunks of 512
F32, BF16 = mybir.dt.float32, mybir.dt.bfloat16

_NC = None


def _build():
    nc = bacc.Bacc()
    # x packed on host as [p][c][kt][t] -> [P, KT*N]
    xP = nc.declare_dram_parameter("xP", [P, KT * N], BF16, isOutput=False)
    # weights packed as [p][kt][e] -> [P, KT*E]
    wlv = nc.declare_dram_parameter("wlv", [P, E * KT], BF16, isOutput=False)
    wmu = nc.declare_dram_parameter("wmu", [P, E * KT], BF16, isOutput=False)
    # biases arranged [P, EB]: element (p, eb) = bias[eb*128 + p]
    bmu = nc.declare_dram_parameter("bmu", [P, EB], F32, isOutput=False)
    blv = nc.declare_dram_parameter("blv", [P, EB], F32, isOutput=False)  # 0.5*b
    # outputs packed [p][eb][c][t]: element (p, eb*N + c*512 + t) =
    # head[c*512 + t, eb*128 + p]
    muT = nc.declare_dram_parameter("muT", [P, EB * N], BF16, isOutput=True)
    sgT = nc.declare_dram_parameter("sgT", [P, EB * N], BF16, isOutput=True)

    with TileContext(nc) as tc:
        with (
            tc.tile_pool(name="const", bufs=1) as cpool,
            tc.tile_pool(name="out", bufs=8) as opool,
            tc.tile_pool(name="psA", bufs=4, space="PSUM") as psA,
            tc.tile_pool(name="psB", bufs=4, space="PSUM") as psB,
        ):
            x_sb = [
                cpool.tile([P, KT, 512], BF16, name=f"x_sb{c}") for c in range(TC)
            ]
            wlv_sb = cpool.tile([P, KT, E], BF16)
            wmu_sb = cpool.tile([P, KT, E], BF16)
            blv_sb = cpool.tile([P, EB], F32)
            bmu_sb = cpool.tile([P, EB], F32)
            warm = cpool.tile([P, P], BF16)
            gate1 = cpool.tile([P, 2], BF16)
            gate2 = cpool.tile([P, 2], BF16)

            wlv_r = wlv[:, :].rearrange("p (kt e) -> p kt e", kt=KT)
            wmu_r = wmu[:, :].rearrange("p (kt e) -> p kt e", kt=KT)

            def xslab(c):
                off = c * 512 * KT
                return xP[:, off : off + 512 * KT].rearrange(
                    "p (kt t) -> p kt t", kt=KT
                )

            # Warmup: PE busy continuously from ~0.2us so the HAM clock
            # gate reaches 2.4 GHz with no re-throttle gap.
            nc.vector.memset(warm, 0)
            wps = psA.tile([P, P], F32, tag="ps", name="warmps")
            for i in range(32):
                nc.tensor.matmul(
                    wps, warm[:, :], warm[:, :], start=(i == 0), stop=(i == 31)
                )

            # --- OPENING: interleaved kt-pairs of x_c0 / wlv ----------
            for kp in range(4):
                s = slice(2 * kp, 2 * kp + 2)
                if kp % 2 == 0:
                    nc.sync.dma_start(out=wlv_sb[:, s, :], in_=wlv_r[:, s, :])
                    nc.scalar.dma_start(out=x_sb[0][:, s, :], in_=xslab(0)[:, s, :])
                else:
                    nc.sync.dma_start(out=x_sb[0][:, s, :], in_=xslab(0)[:, s, :])
                    nc.scalar.dma_start(out=wlv_sb[:, s, :], in_=wlv_r[:, s, :])
            # wmu behind the opening as two kt-half slabs (4KB desc).
            nc.sync.dma_start(out=wmu_sb[:, 0:4, :], in_=wmu_r[:, 0:4, :])
            nc.scalar.dma_start(out=wmu_sb[:, 4:KT, :], in_=wmu_r[:, 4:KT, :])
            # gpsimd: biases now; x_c1 gated behind the first wlv pair;
            # x_c2/x_c3 gated behind x_c1.
            nc.gpsimd.dma_start(out=blv_sb, in_=blv[:, :])
            nc.gpsimd.dma_start(out=bmu_sb, in_=bmu[:, :])
            nc.gpsimd.tensor_copy(gate1, wlv_sb[:, 1, 0:2])
            nc.gpsimd.dma_start(out=x_sb[1], in_=xslab(1))
            nc.gpsimd.tensor_copy(gate2, x_sb[1][:, 7, 0:2])
            nc.gpsimd.dma_start(out=x_sb[2], in_=xslab(2))
            nc.gpsimd.dma_start(out=x_sb[3], in_=xslab(3))

            EXP = mybir.ActivationFunctionType.Exp

            def epilogue(hname, bias_sb, eb, ps, ot, o0, ow):
                """PSUM -> bf16 slice [o0:o0+ow) of the [P, N] out tile."""
                osl = ot[:, o0 : o0 + ow]
                if hname == "lv":
                    nc.scalar.activation(
                        osl, ps, EXP, bias=bias_sb[:, eb : eb + 1], scale=0.5
                    )
                else:
                    nc.vector.tensor_scalar_add(osl, ps, bias_sb[:, eb : eb + 1])

            def store_half(outdram, eb, half, ot, q):
                """Store [P, 1024] column-half (chunks 2h,2h+1) of an eb."""
                ds = slice(eb * N + half * 1024, eb * N + (half + 1) * 1024)
                if q == 0:
                    nc.sync.dma_start(out=outdram[:, ds], in_=ot[:, half * 1024 : (half + 1) * 1024])
                else:
                    nc.scalar.dma_start(out=outdram[:, ds], in_=ot[:, half * 1024 : (half + 1) * 1024])

            # --- lv head: chunk-major, kt-outer -----------------------
            ot_lv = [
                opool.tile([P, N], BF16, tag="o", name=f"o_lv{eb}")
                for eb in range(EB)
            ]
            for c in range(TC):
                pool = [psA, psB][c % 2]
                pss = [
                    pool.tile([P, 512], F32, tag="ps", name=f"ps_lv{c}{eb}")
                    for eb in range(EB)
                ]
                for kt in range(KT):
                    for eb in range(EB):
                        nc.tensor.matmul(
                            pss[eb],
                            wlv_sb[:, kt, eb * P : (eb + 1) * P],
                            x_sb[c][:, kt, :],
                            start=(kt == 0),
                            stop=(kt == KT - 1),
                        )
                for eb in range(EB):
                    epilogue("lv", blv_sb, eb, pss[eb], ot_lv[eb], c * 512, 512)
                if c == 1 or c == 3:
                    for eb in range(EB):
                        store_half(sgT, eb, c // 2, ot_lv[eb], eb % 2)

            # --- mu head: EB-MAJOR, kt-inner; store halves as each
            # half-eb finishes so only eb3's last half trails the end --
            for eb in range(EB):
                ot = opool.tile([P, N], BF16, tag="o", name=f"o_mu{eb}")
                for c in range(TC):
                    pieces = (
                        [(0, 512)]
                        if not (eb == EB - 1 and c == TC - 1)
                        else [(0, 256), (256, 128), (384, 128)]
                    )
                    for o0, ow in pieces:
                        ps = psA.tile(
                            [P, ow], F32, tag="ps", name=f"ps_mu{eb}{c}_{o0}"
                        )
                        for kt in range(KT):
                            nc.tensor.matmul(
                                ps,
                                wmu_sb[:, kt, eb * P : (eb + 1) * P],
                                x_sb[c][:, kt, o0 : o0 + ow],
                                start=(kt == 0),
                                stop=(kt == KT - 1),
                            )
                        epilogue("mu", bmu_sb, eb, ps, ot, c * 512 + o0, ow)
                    if c == 1 or c == 3:
                        store_half(muT, eb, c // 2, ot, (eb + c // 2) % 2)
    nc.compile()
    return nc


def _pack_x(xb):
    """xb [N, D] f32 -> [P, KT*N] bf16 packed as [p][c][kt][t]."""
    xt = xb.T.astype(ml_dtypes.bfloat16).reshape(KT, P, TC, 512)  # [kt, p, c, t]
    return np.ascontiguousarray(xt.transpose(1, 2, 0, 3).reshape(P, KT * N))


def _pack_w(W):
    """W [E, D] f32 -> [P, KT*E] bf16 packed as [p][kt][e]."""
    wt = W.astype(np.float32).T.astype(ml_dtypes.bfloat16)   # [D, E]
    v = wt.reshape(KT, P, E)
    return np.ascontiguousarray(v.transpose(1, 0, 2).reshape(P, KT * E))


def _unpack_out(a):
    """[P, EB*N] bf16 packed [p][eb][c][t] -> [N, E] f32."""
    v = a.reshape(P, EB, N)                      # [p, eb, n]
    return np.ascontiguousarray(v.transpose(2, 1, 0).reshape(N, E)).astype(np.float32)


def run(x, W_mu, b_mu, W_logvar, b_logvar, trace=False, **trace_kwargs):
    global _NC
    if _NC is None:
        _NC = _build()

    x = np.asarray(x, dtype=np.float32)
    wlv_h = _pack_w(np.asarray(W_logvar))
    wmu_h = _pack_w(np.asarray(W_mu))
    bmu_h = np.ascontiguousarray(np.asarray(b_mu, dtype=np.float32).reshape(EB, P).T)
    blv_h = np.ascontiguousarray(
        (0.5 * np.asarray(b_logvar, dtype=np.float32)).reshape(EB, P).T
    )

    in_maps = [
        {
            "xP": _pack_x(x[b]),
            "wlv": wlv_h,
            "wmu": wmu_h,
            "bmu": bmu_h,
            "blv": blv_h,
        }
        for b in range(B)
    ]
    res = run_bass_kernel_spmd(
        _NC, in_maps, core_ids=list(range(B)), trace=trace, **trace_kwargs
    )
    mu = np.stack([_unpack_out(res.results[b]["muT"]) for b in range(B)])
    sigma = np.stack([_unpack_out(res.results[b]["sgT"]) for b in range(B)])
    return (mu, sigma), res


def kernel(x, W_mu, b_mu, W_logvar, b_logvar):
    (mu, sigma), _ = run(x, W_mu, b_mu, W_logvar, b_logvar, trace=False)
    return mu, sigma


# revision 11
# speedup vs baseline: 1.0286x; 1.0095x over previous
"""Trainium2 Bass kernel for GaussianProcessEmbeddingHead.

The reference computes:
    mu     = x @ W_mu.T + b_mu                      (B,N,E)
    sigma  = exp(0.5*(x @ W_logvar.T + b_logvar))   (B,N,E)
    K      = RBF kernel matrix (B,N,N)  -- only its DIAGONAL is used,
             and dist_ii == 0 exactly, so cov_diag == 1 and the (B,N,N)
             work is mathematically dead. sigma_adjusted == sigma.
    return (mu, sigma_adjusted)

Strategy: data-parallel over batch B=8, one batch element per NeuronCore.
Per core: two linear heads over x_b [2048,1024] in bf16. The PE streams
one output column per cycle, so the floor is
   2 heads * (2048*512 outputs / 128 lanes) * (1024/128 k-tiles)
   = 131072 cycles ~= 54.6 us @ 2.4 GHz.

Schedule (v6) — measured DMA laws this is built around:
  * per-queue throughput ~95-105 B/ns when two queues run, ~200 solo;
    gpsimd (SWDGE) adds ~130 on top but starves the HWDGE queues.
  * every DMA's completion semaphore lands ~2us after its last byte
    for 128-partition transfers — but ~4.5us for partition-SPLIT
    (64-row) transfers, so fine-graining is done by k-tile PAIRS and
    column halves, never by partitions.
So:
 - OPENING (the PE-start critical path): x_c0 and wlv stream as
   interleaved kt-pairs on the two HWDGE queues (sync: w01,x23,w45,x67;
   scalar: x01,w23,x45,w67).  Chunk 0 runs kt-OUTER and consumes each
   pair as it lands (~1 pair / 1.7us of compute), so compute starts at
   ~5.5us and chunk 0 co-completes with its own load.
 - x_c1 rides gpsimd, gated behind the first wlv pair (tiny copy) so
   its SWDGE burst doesn't slow the pairs that gate compute start;
   x_c2/x_c3 are gated behind x_c1, needed only at ~26/33us.  wmu
   follows the opening on the HWDGE queues as two kt-half slabs.
 - lv head: chunk-major (kt-outer, 4 PSUM banks); mu head: EB-MAJOR
   (kt-inner groups) so each eb's output completes 6.8us apart.
 - Stores are [128, 1024] column-halves (2 KB descriptors, full 128
   partitions): each (head, eb) stores chunks 01 as soon as c1's
   epilogue lands and chunks 23 at eb end, alternating queues.  After
   the last (tapered 256/128/128) matmul group only ONE 256 KB store
   remains -> post-compute tail ~5us instead of the 11-13us store
   drains of earlier schedules.
 - Warmup matmuls (32) keep the PE busy through the DMA lead-in so the
   HAM clock gate reaches 2.4 GHz with no re-throttle gap.
 - Epilogues: one op per PSUM tile, bias via the per-partition port:
     sigma = Exp(PSUM * 0.5 + 0.5*b_lv[e])  on the Scalar engine
     mu    = PSUM + b_mu[e]                 on the Vector engine
   both writing bf16 into [128, 2048] out tiles ([p][eb][c][t] DRAM
   layout); host un-transposes and upcasts.
"""
import os
import sys

import numpy as np

try:
    import concourse.bass as bass  # noqa: F401
except Exception:  # pragma: no cover - path fallback for fresh dirs
    for p in ("/opt/trn_rl_repo", os.path.expanduser("~/.axon_site/_ro/trn_rl_repo")):
        if os.path.isdir(p) and p not in sys.path:
            sys.path.insert(0, p)
    import concourse.bass as bass

import ml_dtypes
import concourse.mybir as mybir
from concourse import bacc
from concourse.bass_utils import run_bass_kernel_spmd
from concourse.tile import TileContext

B, N, D, E = 8, 2048, 1024, 512
P = 128
KT = D // P          # 8 k-tiles
EB = E // P          # 4 embedding blocks
TC = N // 512        # 4 token chunks of 512
F32, BF16 = mybir.dt.float32, mybir.dt.bfloat16

_NC = None


def _build():
    nc = bacc.Bacc()
    # x packed on host as [p][c][kt][t] -> [P, KT*N]
    xP = nc.declare_dram_parameter("xP", [P, KT * N], BF16, isOutput=False)
    # weights packed as [p][kt][e] -> [P, KT*E]
    wlv = nc.declare_dram_parameter("wlv", [P, E * KT], BF16, isOutput=False)
    wmu = nc.declare_dram_parameter("wmu", [P, E * KT], BF16, isOutput=False)
    # biases arranged [P, EB]: element (p, eb) = bias[eb*128 + p]
    bmu = nc.declare_dram_parameter("bmu", [P, EB], F32, isOutput=False)
    blv = nc.declare_dram_parameter("blv", [P, EB], F32, isOutput=False)  # 0.5*b
    # outputs packed [p][eb][c][t]: element (p, eb*N + c*512 + t) =
    # head[c*512 + t, eb*128 + p]
    muT = nc.declare_dram_parameter("muT", [P, EB * N], BF16, isOutput=True)
    sgT = nc.declare_dram_parameter("sgT", [P, EB * N], BF16, isOutput=True)

    with TileContext(nc) as tc:
        with (
            tc.tile_pool(name="const", bufs=1) as cpool,
            tc.tile_pool(name="out", bufs=8) as opool,
            tc.tile_pool(name="psA", bufs=4, space="PSUM") as psA,
            tc.tile_pool(name="psB", bufs=4, space="PSUM") as psB,
        ):
            x_sb = [
                cpool.tile([P, KT, 512], BF16, name=f"x_sb{c}") for c in range(TC)
            ]
            wlv_sb = cpool.tile([P, KT, E], BF16)
            wmu_sb = cpool.tile([P, KT, E], BF16)
            blv_sb = cpool.tile([P, EB], F32)
            bmu_sb = cpool.tile([P, EB], F32)
            warm = cpool.tile([P, P], BF16)

            wlv_r = wlv[:, :].rearrange("p (kt e) -> p kt e", kt=KT)
            wmu_r = wmu[:, :].rearrange("p (kt e) -> p kt e", kt=KT)

            def xslab(c):
                off = c * 512 * KT
                return xP[:, off : off + 512 * KT].rearrange(
                    "p (kt t) -> p kt t", kt=KT
                )

            # Warmup: PE busy continuously from ~0.2us so the HAM clock
            # gate reaches 2.4 GHz with no re-throttle gap.
            nc.vector.memset(warm, 0)
            wps = psA.tile([P, P], F32, tag="ps", name="warmps")
            for i in range(32):
                nc.tensor.matmul(
                    wps, warm[:, :], warm[:, :], start=(i == 0), stop=(i == 31)
                )

            # --- OPENING -----------------------------------------------
            # gpsimd takes x_c0 whole (8KB descriptors, its fast path),
            # the two HWDGE queues take wlv as kt-halves; all three land
            # ~10.5us and chunk 0 runs dense.  x_c1/wmu kt-halves follow
            # on the HWDGE queues (arrive ~16.5/~22, needed 17.8/32.5);
            # x_c2/x_c3 trail x_c0 on gpsimd (arrive ~18/~26, needed
            # ~24.6/~31.4).  No queue ever starves the opening.
            nc.sync.dma_start(out=wlv_sb[:, 0:4, :], in_=wlv_r[:, 0:4, :])
            nc.scalar.dma_start(out=wlv_sb[:, 4:KT, :], in_=wlv_r[:, 4:KT, :])
            nc.sync.dma_start(out=x_sb[1][:, 0:4, :], in_=xslab(1)[:, 0:4, :])
            nc.scalar.dma_start(out=x_sb[1][:, 4:KT, :], in_=xslab(1)[:, 4:KT, :])
            nc.sync.dma_start(out=wmu_sb[:, 0:4, :], in_=wmu_r[:, 0:4, :])
            nc.scalar.dma_start(out=wmu_sb[:, 4:KT, :], in_=wmu_r[:, 4:KT, :])
            nc.gpsimd.dma_start(out=blv_sb, in_=blv[:, :])
            nc.gpsimd.dma_start(out=bmu_sb, in_=bmu[:, :])
            nc.gpsimd.dma_start(out=x_sb[0], in_=xslab(0))
            nc.gpsimd.dma_start(out=x_sb[2], in_=xslab(2))
            nc.gpsimd.dma_start(out=x_sb[3], in_=xslab(3))

            EXP = mybir.ActivationFunctionType.Exp

            def epilogue(hname, bias_sb, eb, ps, ot, o0, ow):
                """PSUM -> bf16 slice [o0:o0+ow) of the [P, N] out tile."""
                osl = ot[:, o0 : o0 + ow]
                if hname == "lv":
                    nc.scalar.activation(
                        osl, ps, EXP, bias=bias_sb[:, eb : eb + 1], scale=0.5
                    )
                else:
                    nc.vector.tensor_scalar_add(osl, ps, bias_sb[:, eb : eb + 1])

            def store_half(outdram, eb, half, ot, q):
                """Store [P, 1024] column-half (chunks 2h,2h+1) of an eb."""
                ds = slice(eb * N + half * 1024, eb * N + (half + 1) * 1024)
                if q == 0:
                    nc.sync.dma_start(out=outdram[:, ds], in_=ot[:, half * 1024 : (half + 1) * 1024])
                else:
                    nc.scalar.dma_start(out=outdram[:, ds], in_=ot[:, half * 1024 : (half + 1) * 1024])

            # --- lv head: chunk-major, kt-outer -----------------------
            ot_lv = [
                opool.tile([P, N], BF16, tag="o", name=f"o_lv{eb}")
                for eb in range(EB)
            ]
            for c in range(TC):
                pool = [psA, psB][c % 2]
                pss = [
                    pool.tile([P, 512], F32, tag="ps", name=f"ps_lv{c}{eb}")
                    for eb in range(EB)
                ]
                for kt in range(KT):
                    for eb in range(EB):
                        nc.tensor.matmul(
                            pss[eb],
                            wlv_sb[:, kt, eb * P : (eb + 1) * P],
                            x_sb[c][:, kt, :],
                            start=(kt == 0),
                            stop=(kt == KT - 1),
                        )
                for eb in range(EB):
                    epilogue("lv", blv_sb, eb, pss[eb], ot_lv[eb], c * 512, 512)
                if c == 1 or c == 3:
                    for eb in range(EB):
                        store_half(sgT, eb, c // 2, ot_lv[eb], eb % 2)

            # --- mu head: EB-MAJOR, kt-inner; store halves as each
            # half-eb finishes so only eb3's last half trails the end --
            for eb in range(EB):
                ot = opool.tile([P, N], BF16, tag="o", name=f"o_mu{eb}")
                for c in range(TC):
                    pieces = (
                        [(0, 512)]
                        if not (eb == EB - 1 and c == TC - 1)
                        else [(0, 256), (256, 128), (384, 128)]
                    )
                    for o0, ow in pieces:
                        ps = psA.tile(
                            [P, ow], F32, tag="ps", name=f"ps_mu{eb}{c}_{o0}"
                        )
                        for kt in range(KT):
                            nc.tensor.matmul(
                                ps,
                                wmu_sb[:, kt, eb * P : (eb + 1) * P],
                                x_sb[c][:, kt, o0 : o0 + ow],
                                start=(kt == 0),
                                stop=(kt == KT - 1),
                            )
                        epilogue("mu", bmu_sb, eb, ps, ot, c * 512 + o0, ow)
                    if c == 1 or c == 3:
                        store_half(muT, eb, c // 2, ot, (eb + c // 2) % 2)
    nc.compile()
    return nc


def _pack_x(xb):
    """xb [N, D] f32 -> [P, KT*N] bf16 packed as [p][c][kt][t]."""
    xt = xb.T.astype(ml_dtypes.bfloat16).reshape(KT, P, TC, 512)  # [kt, p, c, t]
    return np.ascontiguousarray(xt.transpose(1, 2, 0, 3).reshape(P, KT * N))


def _pack_w(W):
    """W [E, D] f32 -> [P, KT*E] bf16 packed as [p][kt][e]."""
    wt = W.astype(np.float32).T.astype(ml_dtypes.bfloat16)   # [D, E]
    v = wt.reshape(KT, P, E)
    return np.ascontiguousarray(v.transpose(1, 0, 2).reshape(P, KT * E))


def _unpack_out(a):
    """[P, EB*N] bf16 packed [p][eb][c][t] -> [N, E] f32."""
    v = a.reshape(P, EB, N)                      # [p, eb, n]
    return np.ascontiguousarray(v.transpose(2, 1, 0).reshape(N, E)).astype(np.float32)


def run(x, W_mu, b_mu, W_logvar, b_logvar, trace=False, **trace_kwargs):
    global _NC
    if _NC is None:
        _NC = _build()

    x = np.asarray(x, dtype=np.float32)
    wlv_h = _pack_w(np.asarray(W_logvar))
    wmu_h = _pack_w(np.asarray(W_mu))
    bmu_h = np.ascontiguousarray(np.asarray(b_mu, dtype=np.float32).reshape(EB, P).T)
    blv_h = np.ascontiguousarray(
        (0.5 * np.asarray(b_logvar, dtype=np.float32)).reshape(EB, P).T
    )

    in_maps = [
        {
            "xP": _pack_x(x[b]),
            "wlv": wlv_h,
            "wmu": wmu_h,
            "bmu": bmu_h,
            "blv": blv_h,
        }
        for b in range(B)
    ]
    res = run_bass_kernel_spmd(
        _NC, in_maps, core_ids=list(range(B)), trace=trace, **trace_kwargs
    )
    mu = np.stack([_unpack_out(res.results[b]["muT"]) for b in range(B)])
    sigma = np.stack([_unpack_out(res.results[b]["sgT"]) for b in range(B)])
    return (mu, sigma), res


def kernel(x, W_mu, b_mu, W_logvar, b_logvar):
    (mu, sigma), _ = run(x, W_mu, b_mu, W_logvar, b_logvar, trace=False)
    return mu, sigma
